# revision 9
# baseline (speedup 1.0000x reference)
"""AttnAggregator2 Trainium2 kernel — dense-streaming edition, v7.

Math (per node n, with X[n, s, :] = table rows of [self, neigh_0..neigh_24]):
    Q       = table[node] @ Wq^T + bq
    scores  = Q . K  where K = X @ Wk^T + bk
            = (Q @ Wk) . X + (Q . bk)          <- Q.bk cancels in softmax
    attn    = softmax(scores)
    mix     = attn-weighted sum of V = (sum_s attn_s X_s) @ Wv^T + bv

Sharding: data-parallel over nodes, 8 cores. The embedding lookup is resolved
on the host during sharding (SWDGE descgen caps any on-device row-gather at
~10 ns/row); each tile streams densely in fp16.

v7: both free-dim reductions run on the PE as stride-0-output accumulating
matmul chains with a fixed identity stationary (microbench: a saturated PE
queue pipelines fully at ~0.56 ns/row; the ISA caps one matmul's out free
size at 512 elements, hence the chunking):
  - scores[n, s] = sum_{h,k} prod[n, h, s, k]: 16 matmuls, rhs iterates
    (k-outer, s-inner) so the same PSUM cell is revisited every 26 cycles.
  - Xmix[n, d]  = sum_s WG[n, s, d]: 8 matmuls (4-slot chunks of 512).
The v5/v6 DVE+GpSimd halving trees, the 32-wide tensor_reduce, and the v5
26 per-s transpose matmuls are all gone. GpSimd only does the attn 32-wide
expand; softmax runs without max-subtraction (scores bounded ~±30, exp in
fp32 is safe). Output is written transposed [128, n]; host transposes back.
"""

import sys
from contextlib import ExitStack

import numpy as np

sys.path.insert(0, "/opt/trn_rl_repo")

import concourse.bass as bass
import concourse.mybir as mybir
import concourse.tile as tile
from concourse import bacc
from concourse.bass_utils import run_bass_kernel_spmd
from concourse.masks import make_identity

F32 = mybir.dt.float32
F16 = mybir.dt.float16

VOCAB = 100000
N_NODES = 50000
S = 25
S1 = S + 1  # self + sampled neighbors
D = 128
P = 128
N_CORES = 8
N_PER_CORE = N_NODES // N_CORES  # 6250
N_TILES = (N_PER_CORE + P - 1) // P  # 49
N_PAD = N_TILES * P  # 6272
FLAT = S1 * D  # 3328
H = 8  # d-interleave chunks
K = D // H  # 32
KH = K // 2  # 16: k-half per score-reduce matmul (416 out elems <= 512 cap)

# number of WG slots multiplied on GpSimd instead of DVE (load balance)
WG_GPS_SLOTS = 6


def build_kernel(n_tiles: int = N_TILES):
    nc = bacc.Bacc(
        "TRN2",
        target_bir_lowering=False,
        debug=False,
        enable_asserts=False,
    )

    gd = nc.dram_tensor("gd", [n_tiles, P, FLAT], F16, kind="ExternalInput").ap()
    sfT = nc.dram_tensor("sfT", [n_tiles, D, P], F16, kind="ExternalInput").ap()
    wqT = nc.dram_tensor("wqT", [D, D], F16, kind="ExternalInput").ap()
    wk = nc.dram_tensor("wk", [D, D], F16, kind="ExternalInput").ap()
    wvT = nc.dram_tensor("wvT", [D, D], F16, kind="ExternalInput").ap()
    bq = nc.dram_tensor("bq", [D, 1], F32, kind="ExternalInput").ap()
    bv = nc.dram_tensor("bv", [D, 1], F32, kind="ExternalInput").ap()
    out = nc.dram_tensor("out", [D, n_tiles * P], F32, kind="ExternalOutput").ap()

    with tile.TileContext(nc) as tc, ExitStack() as ctx:
        const = ctx.enter_context(tc.tile_pool(name="const", bufs=1))
        gpool = ctx.enter_context(tc.tile_pool(name="gpool", bufs=4))
        sfp = ctx.enter_context(tc.tile_pool(name="sfp", bufs=4))
        prodp = ctx.enter_context(tc.tile_pool(name="prodp", bufs=3))
        wgp = ctx.enter_context(tc.tile_pool(name="wgp", bufs=3))
        small = ctx.enter_context(tc.tile_pool(name="small", bufs=8))
        outp = ctx.enter_context(tc.tile_pool(name="outp", bufs=3))
        # PSUM: 8 banks total; 4 tiles x 2 bufs (ps_q is reused for the
        # qT, qp and out-projection matmuls of one tile)
        psA = ctx.enter_context(tc.tile_pool(name="psA", bufs=2, space="PSUM"))
        psB = ctx.enter_context(tc.tile_pool(name="psB", bufs=2, space="PSUM"))
        psC = ctx.enter_context(tc.tile_pool(name="psC", bufs=2, space="PSUM"))
        psD = ctx.enter_context(tc.tile_pool(name="psD", bufs=2, space="PSUM"))

        ident = const.tile([P, P], F32)
        make_identity(nc, ident[:])
        ident16 = const.tile([P, P], F16)
        nc.scalar.copy(ident16[:], ident[:])
        wqT_s = const.tile([D, D], F16)
        nc.sync.dma_start(wqT_s[:], wqT)
        wk_s = const.tile([D, D], F16)
        nc.sync.dma_start(wk_s[:], wk)
        wvT_s = const.tile([D, D], F16)
        nc.sync.dma_start(wvT_s[:], wvT)
        bq_s = const.tile([D, 1], F32)
        nc.sync.dma_start(bq_s[:], bq)
        bv_s = const.tile([D, 1], F32)
        nc.sync.dma_start(bv_s[:], bv)

        for t in range(n_tiles):
            # Dense loads: interleaved rows G and XselfT
            g = gpool.tile([P, FLAT], F16)
            nc.sync.dma_start(g[:], gd[t])
            g4 = g[:].rearrange("p (h s k) -> p h s k", h=H, s=S1, k=K)
            xsT = sfp.tile([P, P], F16)
            nc.sync.dma_start(xsT[:], sfT[t])

            # Q^T = Wq @ Xself^T + bq   [j, n]
            ps_q = psA.tile([P, P], F32)
            nc.tensor.matmul(ps_q[:], lhsT=wqT_s[:], rhs=xsT[:], start=True, stop=True)
            qT = small.tile([P, P], F16)
            nc.scalar.activation(
                qT[:],
                ps_q[:],
                func=mybir.ActivationFunctionType.Identity,
                bias=bq_s[:, :1],
            )

            # Q' = Q @ Wk   [n, d]  (lhsT = Q^T; reuses the same PSUM bank)
            nc.tensor.matmul(ps_q[:], lhsT=qT[:], rhs=wk_s[:], start=True, stop=True)
            qp = small.tile([P, P], F16)
            nc.scalar.copy(qp[:], ps_q[:])
            qp4 = qp[:].rearrange("p (h k) -> p h k", h=H, k=K)

            # prod[n, h, s, k] = G * Q'-broadcast (DVE 2x)
            prod = prodp.tile([P, FLAT], F16)
            nc.vector.tensor_tensor(
                prod[:].rearrange("p (h s k) -> p h s k", h=H, s=S1, k=K),
                g4,
                qp4[:, :, None, :].to_broadcast([P, H, S1, K]),
                op=mybir.AluOpType.mult,
            )

            # scores[n, s] = sum_{h,k} prod: PE accumulator, 16 matmuls of
            # 416 rows; rhs iterates k-outer/s-inner so each PSUM cell is
            # revisited every 26 cycles.
            prod_ks = prod[:].rearrange("p (h s k) -> p h k s", h=H, s=S1, k=K)
            ps_sc = psB.tile([P, S1], F32)
            sc_out = ps_sc[:][:, None, :].to_broadcast([P, KH, S1])
            step = 0
            for h in range(H):
                for kh in range(2):
                    nc.tensor.matmul(
                        sc_out,
                        lhsT=ident16[:],
                        rhs=prod_ks[:, h, kh * KH : (kh + 1) * KH, :],
                        start=(step == 0),
                        stop=(step == 2 * H - 1),
                        skip_group_check=(step > 0),
                    )
                    step += 1

            # softmax over s: scores bounded (|s| < ~35), exp in fp32 is safe
            e = small.tile([P, S1], F32)
            zsum = small.tile([P, 1], F32)
            nc.scalar.activation(
                e[:],
                ps_sc[:],
                func=mybir.ActivationFunctionType.Exp,
                accum_out=zsum[:],
            )
            zinv = small.tile([P, 1], F32)
            nc.vector.reciprocal(zinv[:], zsum[:])
            attn = small.tile([P, S1], F16)
            nc.vector.tensor_tensor(
                attn[:],
                e[:],
                zinv[:].to_broadcast([P, S1]),
                op=mybir.AluOpType.mult,
            )
            # expand attn to 32 per slot (GpSimd; keeps ACT/DVE free)
            a32 = small.tile([P, S1, K], F16)
            nc.gpsimd.tensor_copy(a32[:], attn[:, :, None].to_broadcast([P, S1, K]))

            # WG = G * attn  (written de-interleaved [P, s, d]; split
            # DVE / GpSimd for load balance)
            ns_dve = S1 - WG_GPS_SLOTS
            wg = wgp.tile([P, S1, D], F16)
            nc.vector.tensor_tensor(
                wg[:, :ns_dve, :].rearrange("p s (h k) -> p h s k", h=H, k=K),
                g4[:, :, :ns_dve, :],
                a32[:, None, :ns_dve, :].to_broadcast([P, H, ns_dve, K]),
                op=mybir.AluOpType.mult,
            )
            if WG_GPS_SLOTS:
                nc.gpsimd.tensor_tensor(
                    wg[:, ns_dve:, :].rearrange("p s (h k) -> p h s k", h=H, k=K),
                    g4[:, :, ns_dve:, :],
                    a32[:, None, ns_dve:, :].to_broadcast([P, H, WG_GPS_SLOTS, K]),
                    op=mybir.AluOpType.mult,
                )

            # Xmix[n, d] = sum_s WG[n, s, d]: PE accumulator, 2 singles +
            # 6 four-slot stride-0 chunks (512-elem ISA cap per matmul)
            ps_xm = psC.tile([P, P], F32)
            nc.tensor.matmul(
                ps_xm[:], lhsT=ident16[:], rhs=wg[:, 0, :], start=True, stop=False
            )
            nc.tensor.matmul(
                ps_xm[:], lhsT=ident16[:], rhs=wg[:, 1, :],
                start=False, stop=False, skip_group_check=True,
            )
            for c in range(6):
                nc.tensor.matmul(
                    ps_xm[:][:, None, :].to_broadcast([P, 4, P]),
                    lhsT=ident16[:],
                    rhs=wg[:, 2 + 4 * c : 6 + 4 * c, :],
                    start=False, stop=(c == 5),
                    skip_group_check=True,
                )
            xm16 = small.tile([P, P], F16)
            nc.scalar.copy(xm16[:], ps_xm[:])

            # Xmix^T via one PE transpose
            ps_tr = psD.tile([P, P], F16)
            nc.tensor.transpose(ps_tr[:], xm16[:], ident16[:])
            tr16 = small.tile([P, P], F16)
            nc.scalar.copy(tr16[:], ps_tr[:])

            # out^T = Wv @ Xmix^T + bv   [j, n]  (reuses ps_q's bank)
            nc.tensor.matmul(ps_q[:], lhsT=wvT_s[:], rhs=tr16[:], start=True, stop=True)
            o_t = outp.tile([P, P], F32)
            nc.scalar.activation(
                o_t[:],
                ps_q[:],
                func=mybir.ActivationFunctionType.Identity,
                bias=bv_s[:, :1],
            )
            nc.sync.dma_start(out[:, bass.ts(t, P)], o_t[:])

    nc.compile()
    return nc


_NC_CACHE = {}


def _get_nc():
    key = N_TILES
    if key not in _NC_CACHE:
        _NC_CACHE[key] = build_kernel()
    return _NC_CACHE[key]


def prepare_in_maps(inputs: dict) -> list[dict]:
    """Shard FULL inputs into per-core input maps (host resolves the lookups)."""
    table = np.asarray(inputs["table"], dtype=np.float32)
    node = np.asarray(inputs["node"]).astype(np.int64)
    neighs = np.asarray(inputs["neighs"]).astype(np.int64)
    Wq = np.asarray(inputs["Wq"], dtype=np.float32)
    bq = np.asarray(inputs["bq"], dtype=np.float32)
    Wk = np.asarray(inputs["Wk"], dtype=np.float32)
    Wv = np.asarray(inputs["Wv"], dtype=np.float32)
    bv = np.asarray(inputs["bv"], dtype=np.float32)

    table16 = table.astype(np.float16)
    idx_full = np.concatenate([node[:, None], neighs], axis=1)  # [N, S1]

    common = {
        "wqT": np.ascontiguousarray(Wq.T.astype(np.float16)),
        "wk": np.ascontiguousarray(Wk.astype(np.float16)),
        "wvT": np.ascontiguousarray(Wv.T.astype(np.float16)),
        "bq": np.ascontiguousarray(bq[:, None]),
        "bv": np.ascontiguousarray(bv[:, None]),
    }

    in_maps = []
    for c in range(N_CORES):
        idx_c = idx_full[c * N_PER_CORE : (c + 1) * N_PER_CORE]
        idx_pad = np.zeros((N_PAD, S1), dtype=np.int64)
        idx_pad[:N_PER_CORE] = idx_c
        gfull = table16[idx_pad]  # [N_PAD, S1, D] fp16
        sfT_arr = np.ascontiguousarray(
            gfull[:, 0, :].reshape(N_TILES, P, D).transpose(0, 2, 1)
        )  # [N_TILES, D, P]
        # d-interleave: flat = (d//K)*S1*K + s*K + d%K
        gi = (
            gfull.reshape(N_PAD, S1, H, K)
            .transpose(0, 2, 1, 3)
            .reshape(N_TILES, P, FLAT)
        )
        in_maps.append(
            dict(common, gd=np.ascontiguousarray(gi), sfT=sfT_arr)
        )
    return in_maps


def kernel(**inputs) -> np.ndarray:
    in_maps = prepare_in_maps(inputs)
    nc = _get_nc()
    results = run_bass_kernel_spmd(nc, in_maps, list(range(N_CORES))).results

    out = np.empty((N_NODES, D), dtype=np.float32)
    for c in range(N_CORES):
        out[c * N_PER_CORE : (c + 1) * N_PER_CORE] = results[c]["out"][
            :, :N_PER_CORE
        ].T
    return out


if __name__ == "__main__":
    rng = np.random.default_rng(0)
    inputs = {
        "table": rng.standard_normal((VOCAB, D), dtype=np.float32),
        "node": rng.integers(0, VOCAB, (N_NODES,)),
        "neighs": rng.integers(0, VOCAB, (N_NODES, S)),
        "Wq": rng.uniform(-0.09, 0.09, (D, D)).astype(np.float32),
        "bq": rng.uniform(-0.09, 0.09, (D,)).astype(np.float32),
        "Wk": rng.uniform(-0.09, 0.09, (D, D)).astype(np.float32),
        "bk": rng.uniform(-0.09, 0.09, (D,)).astype(np.float32),
        "Wv": rng.uniform(-0.09, 0.09, (D, D)).astype(np.float32),
        "bv": rng.uniform(-0.09, 0.09, (D,)).astype(np.float32),
    }
    res = kernel(**inputs)
    print("kernel ran, output shape", res.shape)


# revision 10
# speedup vs baseline: 1.8358x; 1.8358x over previous
"""AttnAggregator2 Trainium2 kernel — dense-streaming edition, v8.

Math (per node n, with X[n, s, :] = table rows of [self, neigh_0..neigh_24]):
    Q       = table[node] @ Wq^T + bq
    scores  = Q . K  where K = X @ Wk^T + bk
            = (Q @ Wk) . X + (Q . bk)          <- Q.bk cancels in softmax
    attn    = softmax(scores)
    mix     = attn-weighted sum of V = (sum_s attn_s X_s) @ Wv^T + bv

Sharding: data-parallel over nodes, 8 cores. The embedding lookup is resolved
on the host during sharding (SWDGE descgen caps any on-device row-gather at
~10 ns/row); each tile streams densely in fp16.

v8 (lessons from v5-v7 traces):
  - PE moving operands must be CONTIGUOUS (strided rhs ran at ~3.2 ns/row
    vs ~0.56 saturated-contiguous); a saturated PE queue hides per-inst
    overhead entirely.
  - GpSimd measured 2.3-4.6 ns/elem under contention — removed entirely.
  - scores: prod = G * Q' (DVE 2x), then PE accumulates the 8 d-chunks
    (8 contiguous 416-row matmuls -> PSUM [P, S1, 16], injective), then a
    single DVE 16-wide tensor_reduce -> [P, S1].
  - weighted sum: 26 slots of WG accumulate on PE via stride-0-output
    chunks of 4 slots (512-elem ISA cap; PSUM cell revisit every 128
    cycles is hazard-free, verified in v6).
  - softmax without max-subtraction (scores bounded ~±30; fp32 exp safe).
Output is written transposed [128, n]; host transposes back.
"""

import sys
from contextlib import ExitStack

import numpy as np

sys.path.insert(0, "/opt/trn_rl_repo")

import concourse.bass as bass
import concourse.mybir as mybir
import concourse.tile as tile
from concourse import bacc
from concourse.bass_utils import run_bass_kernel_spmd
from concourse.masks import make_identity

F32 = mybir.dt.float32
F16 = mybir.dt.float16

VOCAB = 100000
N_NODES = 50000
S = 25
S1 = S + 1  # self + sampled neighbors
D = 128
P = 128
N_CORES = 8
N_PER_CORE = N_NODES // N_CORES  # 6250
N_TILES = (N_PER_CORE + P - 1) // P  # 49
N_PAD = N_TILES * P  # 6272
FLAT = S1 * D  # 3328
H = 8  # d-interleave chunks
K = D // H  # 16


def build_kernel(n_tiles: int = N_TILES):
    nc = bacc.Bacc(
        "TRN2",
        target_bir_lowering=False,
        debug=False,
        enable_asserts=False,
    )

    gd = nc.dram_tensor("gd", [n_tiles, P, FLAT], F16, kind="ExternalInput").ap()
    sfT = nc.dram_tensor("sfT", [n_tiles, D, P], F16, kind="ExternalInput").ap()
    wqT = nc.dram_tensor("wqT", [D, D], F16, kind="ExternalInput").ap()
    wk = nc.dram_tensor("wk", [D, D], F16, kind="ExternalInput").ap()
    wvT = nc.dram_tensor("wvT", [D, D], F16, kind="ExternalInput").ap()
    bq = nc.dram_tensor("bq", [D, 1], F32, kind="ExternalInput").ap()
    bv = nc.dram_tensor("bv", [D, 1], F32, kind="ExternalInput").ap()
    out = nc.dram_tensor("out", [D, n_tiles * P], F32, kind="ExternalOutput").ap()

    with tile.TileContext(nc) as tc, ExitStack() as ctx:
        const = ctx.enter_context(tc.tile_pool(name="const", bufs=1))
        gpool = ctx.enter_context(tc.tile_pool(name="gpool", bufs=4))
        sfp = ctx.enter_context(tc.tile_pool(name="sfp", bufs=4))
        prodp = ctx.enter_context(tc.tile_pool(name="prodp", bufs=3))
        wgp = ctx.enter_context(tc.tile_pool(name="wgp", bufs=3))
        small = ctx.enter_context(tc.tile_pool(name="small", bufs=10))
        outp = ctx.enter_context(tc.tile_pool(name="outp", bufs=3))
        # PSUM: 8 banks; 4 pools x 2 bufs x 1 bank each (ps_q is reused for
        # the qT, qp and out-projection matmuls of one tile)
        psA = ctx.enter_context(tc.tile_pool(name="psA", bufs=2, space="PSUM"))
        psB = ctx.enter_context(tc.tile_pool(name="psB", bufs=2, space="PSUM"))
        psC = ctx.enter_context(tc.tile_pool(name="psC", bufs=2, space="PSUM"))
        psD = ctx.enter_context(tc.tile_pool(name="psD", bufs=2, space="PSUM"))

        ident = const.tile([P, P], F32)
        make_identity(nc, ident[:])
        ident16 = const.tile([P, P], F16)
        nc.scalar.copy(ident16[:], ident[:])
        wqT_s = const.tile([D, D], F16)
        nc.sync.dma_start(wqT_s[:], wqT)
        wk_s = const.tile([D, D], F16)
        nc.sync.dma_start(wk_s[:], wk)
        wvT_s = const.tile([D, D], F16)
        nc.sync.dma_start(wvT_s[:], wvT)
        bq_s = const.tile([D, 1], F32)
        nc.sync.dma_start(bq_s[:], bq)
        bv_s = const.tile([D, 1], F32)
        nc.sync.dma_start(bv_s[:], bv)

        for t in range(n_tiles):
            # Dense loads: interleaved rows G and XselfT
            g = gpool.tile([P, FLAT], F16)
            nc.sync.dma_start(g[:], gd[t])
            g4 = g[:].rearrange("p (h s k) -> p h s k", h=H, s=S1, k=K)
            xsT = sfp.tile([P, P], F16)
            nc.sync.dma_start(xsT[:], sfT[t])

            # Q^T = Wq @ Xself^T + bq   [j, n]
            ps_q = psA.tile([P, P], F32)
            nc.tensor.matmul(ps_q[:], lhsT=wqT_s[:], rhs=xsT[:], start=True, stop=True)
            qT = small.tile([P, P], F16)
            nc.scalar.activation(
                qT[:],
                ps_q[:],
                func=mybir.ActivationFunctionType.Identity,
                bias=bq_s[:, :1],
            )

            # Q' = Q @ Wk   [n, d]  (lhsT = Q^T; reuses the same PSUM bank)
            nc.tensor.matmul(ps_q[:], lhsT=qT[:], rhs=wk_s[:], start=True, stop=True)
            qp = small.tile([P, P], F16)
            nc.scalar.copy(qp[:], ps_q[:])
            qp4 = qp[:].rearrange("p (h k) -> p h k", h=H, k=K)

            # prod[n, h, s, k] = G * Q'-broadcast (DVE 2x)
            prod = prodp.tile([P, FLAT], F16)
            prod4 = prod[:].rearrange("p (h s k) -> p h s k", h=H, s=S1, k=K)
            nc.vector.tensor_tensor(
                prod4,
                g4,
                qp4[:, :, None, :].to_broadcast([P, H, S1, K]),
                op=mybir.AluOpType.mult,
            )

            # partial scores: PE accumulates the 8 d-chunks (contiguous
            # 416-row matmuls) -> ps_hs[n, s, k]
            ps_hs = psB.tile([P, S1, K], F32)
            for h in range(H):
                nc.tensor.matmul(
                    ps_hs[:],
                    lhsT=ident16[:],
                    rhs=prod4[:, h],
                    start=(h == 0),
                    stop=(h == H - 1),
                    skip_group_check=(h > 0),
                )
            # scores[n, s]: 16-wide reduce (DVE, reads PSUM)
            sc = small.tile([P, S1], F32)
            nc.vector.tensor_reduce(
                sc[:],
                ps_hs[:],
                axis=mybir.AxisListType.X,
                op=mybir.AluOpType.add,
            )

            # softmax over s: scores bounded (|s| < ~35), exp in fp32 is safe
            e = small.tile([P, S1], F32)
            zsum = small.tile([P, 1], F32)
            nc.scalar.activation(
                e[:],
                sc[:],
                func=mybir.ActivationFunctionType.Exp,
                accum_out=zsum[:],
            )
            zinv = small.tile([P, 1], F32)
            nc.vector.reciprocal(zinv[:], zsum[:])
            attn = small.tile([P, S1], F16)
            nc.vector.tensor_tensor(
                attn[:],
                e[:],
                zinv[:].to_broadcast([P, S1]),
                op=mybir.AluOpType.mult,
            )
            # expand attn to K per slot so the weighting multiply is 2x
            a32 = small.tile([P, S1, K], F16)
            nc.scalar.copy(a32[:], attn[:, :, None].to_broadcast([P, S1, K]))

            # WG = G * attn  (DVE 2x, written de-interleaved [P, s, d])
            wg = wgp.tile([P, S1, D], F16)
            nc.vector.tensor_tensor(
                wg[:].rearrange("p s (h k) -> p h s k", h=H, k=K),
                g4,
                a32[:, None, :, :].to_broadcast([P, H, S1, K]),
                op=mybir.AluOpType.mult,
            )

            # Xmix[n, d] = sum_s WG[n, s, d]: PE accumulator, 2 singles +
            # 6 four-slot stride-0 chunks (512-elem ISA cap per matmul)
            ps_xm = psC.tile([P, P], F32)
            nc.tensor.matmul(
                ps_xm[:], lhsT=ident16[:], rhs=wg[:, 0, :], start=True, stop=False
            )
            nc.tensor.matmul(
                ps_xm[:], lhsT=ident16[:], rhs=wg[:, 1, :],
                start=False, stop=False, skip_group_check=True,
            )
            for c in range(6):
                nc.tensor.matmul(
                    ps_xm[:][:, None, :].to_broadcast([P, 4, P]),
                    lhsT=ident16[:],
                    rhs=wg[:, 2 + 4 * c : 6 + 4 * c, :],
                    start=False, stop=(c == 5),
                    skip_group_check=True,
                )
            xm16 = small.tile([P, P], F16)
            nc.scalar.copy(xm16[:], ps_xm[:])

            # Xmix^T via one PE transpose
            ps_tr = psD.tile([P, P], F16)
            nc.tensor.transpose(ps_tr[:], xm16[:], ident16[:])
            tr16 = small.tile([P, P], F16)
            nc.scalar.copy(tr16[:], ps_tr[:])

            # out^T = Wv @ Xmix^T + bv   [j, n]  (reuses ps_q's bank)
            nc.tensor.matmul(ps_q[:], lhsT=wvT_s[:], rhs=tr16[:], start=True, stop=True)
            o_t = outp.tile([P, P], F32)
            nc.scalar.activation(
                o_t[:],
                ps_q[:],
                func=mybir.ActivationFunctionType.Identity,
                bias=bv_s[:, :1],
            )
            nc.sync.dma_start(out[:, bass.ts(t, P)], o_t[:])

    nc.compile()
    return nc


_NC_CACHE = {}


def _get_nc():
    key = N_TILES
    if key not in _NC_CACHE:
        _NC_CACHE[key] = build_kernel()
    return _NC_CACHE[key]


def prepare_in_maps(inputs: dict) -> list[dict]:
    """Shard FULL inputs into per-core input maps (host resolves the lookups)."""
    table = np.asarray(inputs["table"], dtype=np.float32)
    node = np.asarray(inputs["node"]).astype(np.int64)
    neighs = np.asarray(inputs["neighs"]).astype(np.int64)
    Wq = np.asarray(inputs["Wq"], dtype=np.float32)
    bq = np.asarray(inputs["bq"], dtype=np.float32)
    Wk = np.asarray(inputs["Wk"], dtype=np.float32)
    Wv = np.asarray(inputs["Wv"], dtype=np.float32)
    bv = np.asarray(inputs["bv"], dtype=np.float32)

    table16 = table.astype(np.float16)
    idx_full = np.concatenate([node[:, None], neighs], axis=1)  # [N, S1]

    common = {
        "wqT": np.ascontiguousarray(Wq.T.astype(np.float16)),
        "wk": np.ascontiguousarray(Wk.astype(np.float16)),
        "wvT": np.ascontiguousarray(Wv.T.astype(np.float16)),
        "bq": np.ascontiguousarray(bq[:, None]),
        "bv": np.ascontiguousarray(bv[:, None]),
    }

    in_maps = []
    for c in range(N_CORES):
        idx_c = idx_full[c * N_PER_CORE : (c + 1) * N_PER_CORE]
        idx_pad = np.zeros((N_PAD, S1), dtype=np.int64)
        idx_pad[:N_PER_CORE] = idx_c
        gfull = table16[idx_pad]  # [N_PAD, S1, D] fp16
        sfT_arr = np.ascontiguousarray(
            gfull[:, 0, :].reshape(N_TILES, P, D).transpose(0, 2, 1)
        )  # [N_TILES, D, P]
        # d-interleave: flat = (d//K)*S1*K + s*K + d%K
        gi = (
            gfull.reshape(N_PAD, S1, H, K)
            .transpose(0, 2, 1, 3)
            .reshape(N_TILES, P, FLAT)
        )
        in_maps.append(
            dict(common, gd=np.ascontiguousarray(gi), sfT=sfT_arr)
        )
    return in_maps


def kernel(**inputs) -> np.ndarray:
    in_maps = prepare_in_maps(inputs)
    nc = _get_nc()
    results = run_bass_kernel_spmd(nc, in_maps, list(range(N_CORES))).results

    out = np.empty((N_NODES, D), dtype=np.float32)
    for c in range(N_CORES):
        out[c * N_PER_CORE : (c + 1) * N_PER_CORE] = results[c]["out"][
            :, :N_PER_CORE
        ].T
    return out


if __name__ == "__main__":
    rng = np.random.default_rng(0)
    inputs = {
        "table": rng.standard_normal((VOCAB, D), dtype=np.float32),
        "node": rng.integers(0, VOCAB, (N_NODES,)),
        "neighs": rng.integers(0, VOCAB, (N_NODES, S)),
        "Wq": rng.uniform(-0.09, 0.09, (D, D)).astype(np.float32),
        "bq": rng.uniform(-0.09, 0.09, (D,)).astype(np.float32),
        "Wk": rng.uniform(-0.09, 0.09, (D, D)).astype(np.float32),
        "bk": rng.uniform(-0.09, 0.09, (D,)).astype(np.float32),
        "Wv": rng.uniform(-0.09, 0.09, (D, D)).astype(np.float32),
        "bv": rng.uniform(-0.09, 0.09, (D,)).astype(np.float32),
    }
    res = kernel(**inputs)
    print("kernel ran, output shape", res.shape)
